# revision 12
# baseline (speedup 1.0000x reference)
"""Trainium2 Bass kernel for the CellLoss problem.

loss = mean_i [ 1/(x[i, l_i] + 0.1) + sum_j x[i,j] * (x[i,j] > x[i, l_i]) ]
with x: [131072, 256] f32, l: [131072] int labels in [0, 256).

Pure data parallel across 8 NeuronCores (16384 rows each). Per core,
row r lives on partition r % 128; tile t is rows [t*128, (t+1)*128).

Key structure (v3):
  gather: two-stage. Stage 1 uses gpsimd.dma_gather to fetch, for every
      row, the 64-float (256 B) window of the row that contains its
      label element (window index r*4 + l//64, int16, so 4 calls of
      4096 windows each). Stage 2 extracts the label element from each
      64-wide window with a DVE stt (iota64 == l%64)*w row-accumulated
      into G — a 64-element pass instead of the 256-element pass a
      direct gather needs (~259 ns vs ~569 ns per tile).
  margin, two engine variants cycled by PATTERN per 16-tile chunk:
   "D": DVE stt, (x is_gt g) mult x with row-sum accumulator into M.
   "A": scalar-engine Relu(x-g) and Sign(g-x) passes writing bf16 tiles;
      the tensor engine accumulates GLOBAL sums in PSUM: ones^T @
      relu-pairs, and gb^T @ sign(g-x) per tile (gb = bf16(g), so the
      matmuls run at bf16 rate, 4x the f32 rate). Using
      sum_i g_i*cnt_i = (-sum g*sign(g-x) + 255*sum g)/2 (sign(0)=0 at
      the label), the margin needs only these global sums.
Tail: inv = 1/(G+0.1); per-row totals + the A-tile 127.5*g correction;
partition sum via ones-matmul; one f32 partial per core; the host sums
the 8 partials and divides by B.

bf16 is used ONLY for relu magnitudes (unbiased rounding, ~1e-6 effect),
the sign values (exact), and the g matmul weights for the count term
(~1e-5 effect); G itself stays exact f32 for the inv term and compares.

This walrus accepts one sync wait per instruction; Tile can emit
several. _split_multi_waits() hoists extras onto Drain carriers.
"""

import numpy as np
from contextlib import ExitStack

import concourse.bass as bass
import concourse.mybir as mybir
import concourse.tile as tile
from concourse.bass_utils import run_bass_kernel_spmd

F32 = mybir.dt.float32
BF16 = mybir.dt.bfloat16
I16 = mybir.dt.int16

B, C = 131072, 256
N_CORES = 8
B_LOCAL = B // N_CORES          # 16384
P = 128
N_TILES = B_LOCAL // P          # 128
TILES_PER_DMA = 16              # [128, 4096] f32 = 2 MiB per DMA
N_CHUNKS = N_TILES // TILES_PER_DMA
W = 64                          # window elements (256 B, dma_gather minimum)
WPR = C // W                    # windows per row = 4
N_GCALLS = 4                    # dma_gather calls (int16 index limit)
ROWS_PER_GCALL = B_LOCAL // N_GCALLS    # 4096
TILES_PER_GCALL = N_TILES // N_GCALLS   # 32

# margin engine per tile within each 16-tile chunk ("D" DVE stt /
# "A" scalar engine + tensor engine); "A" count per chunk must be even
# (pairs share one bf16 relu tile for the ones-matmul)
PATTERN = list("AAAAAAAAAADDDDDD")

_NC_CACHE = {}
LAST_RESULTS = None
SPLIT_WAITS = True   # off for CoreSim (its event loop rejects bare Drains)
TRACE = False
TRACE_KW = {}
DEBUG_G = False      # add a gout output carrying the gathered G tile


def _split_multi_waits(nc):
    for f in nc.m.functions:
        for blk in f.blocks:
            insts = list(blk.instructions)
            out = []
            changed = False
            for inst in insts:
                si = inst.sync_info
                if si is not None and si.on_wait is not None and len(si.on_wait) > 1:
                    waits = list(si.on_wait)
                    for w in waits[:-1]:
                        d = mybir.InstDrain(
                            name=nc.get_next_instruction_name(),
                            ins=[], outs=[], bass_is_fusable=False)
                        d.engine = inst.engine
                        d.sync_info = mybir.SyncInfo(on_wait=[w], on_update=[])
                        out.append(d)
                    inst.sync_info = mybir.SyncInfo(
                        on_wait=[waits[-1]], on_update=list(si.on_update or []))
                    changed = True
                out.append(inst)
            if changed:
                blk.instructions = out


def _assignment():
    assert N_TILES % len(PATTERN) == 0
    return [PATTERN[t % len(PATTERN)] for t in range(N_TILES)]


def build_nc():
    key = (tuple(_assignment()), SPLIT_WAITS, DEBUG_G)
    if key in _NC_CACHE:
        return _NC_CACHE[key]

    assign = _assignment()
    a_tiles = [t for t, c in enumerate(assign) if c == "A"]
    n_a = len(a_tiles)
    for c in range(N_CHUNKS):
        n_ac = sum(1 for t in range(c * TILES_PER_DMA, (c + 1) * TILES_PER_DMA)
                   if assign[t] == "A")
        assert n_ac % 2 == 0, "A count per chunk must be even"

    nc = bass.Bass()
    x = nc.declare_dram_parameter("x", [B_LOCAL, C], F32, isOutput=False)
    lbl = nc.declare_dram_parameter("lbl", [P, N_TILES], F32, isOutput=False)
    out = nc.declare_dram_parameter("out", [1, 1], F32, isOutput=True)
    gout = (nc.declare_dram_parameter("gout", [P, N_TILES], F32, isOutput=True)
            if DEBUG_G else None)

    # row r = p*128 + t  ->  partition p, tile t
    xv = x.rearrange("(p t) c -> p (t c)", p=P, t=N_TILES)

    with tile.TileContext(nc) as tc, ExitStack() as ctx:
        singles = ctx.enter_context(tc.tile_pool(name="singles", bufs=1))
        xpool = ctx.enter_context(tc.tile_pool(name="x", bufs=4))
        scr = ctx.enter_context(tc.tile_pool(name="scr", bufs=4))
        prs = ctx.enter_context(tc.tile_pool(name="prs", bufs=4))
        psum = ctx.enter_context(tc.tile_pool(name="psum", bufs=1, space="PSUM"))

        HMB = TILES_PER_DMA // 2 * C   # half-chunk elements per partition

        def issue_xw(chunk):
            xw = xpool.tile([P, TILES_PER_DMA * C], F32, name="xw")
            base = chunk * TILES_PER_DMA * C
            for h2 in (0, 1):
                nc.sync.dma_start(
                    xw[:, h2 * HMB:(h2 + 1) * HMB],
                    xv[:, base + h2 * HMB:base + (h2 + 1) * HMB])
            return xw

        xw0 = issue_xw(0)

        lbl_sb = singles.tile([P, N_TILES], F32)
        nc.sync.dma_start(lbl_sb[:], lbl[:])

        iota_i = singles.tile([P, C], mybir.dt.int32)
        nc.gpsimd.iota(iota_i[:], pattern=[[1, C]], base=0, channel_multiplier=0)
        iota_f = singles.tile([P, C], F32)
        nc.vector.tensor_copy(iota_f[:], iota_i[:])

        ones = singles.tile([P, 1], F32)
        nc.vector.memset(ones[:], 1.0)

        G = singles.tile([P, N_TILES], F32)
        M = singles.tile([P, N_TILES], F32)      # D-tile margins; A cols = 0
        if n_a:
            ones_bf = singles.tile([P, 1], BF16)
            nc.vector.memset(ones_bf[:], 1.0)
            nc.vector.memset(M[:], 0.0)
            NG = singles.tile([P, N_TILES], F32)   # -g (f32, ACT bias)
            GB = singles.tile([P, N_TILES], BF16)  # bf16 g (matmul weights)
            ps_r = psum.tile([P, 512], F32, tag="ps_r")
            ps_s = [psum.tile([P, 512], F32, tag=f"ps_s{i}", name=f"ps_s{i}")
                    for i in range(2)]

        mm_r = 0
        mm_s = [0, 0]
        a_seen = 0
        rpair = None
        for chunk in range(N_CHUNKS):
            xw = xw0 if chunk == 0 else issue_xw(chunk)
            HALF = TILES_PER_DMA // 2
            d_tiles = []
            for h in range(2):
                ks = range(h * HALF, (h + 1) * HALF)
                for kk in ks:
                    t = chunk * TILES_PER_DMA + kk
                    xb = xw[:, kk * C:(kk + 1) * C]
                    sel = scr.tile([P, C], F32, tag="sel", name="sel")
                    nc.vector.scalar_tensor_tensor(
                        out=sel[:], in0=iota_f[:], scalar=lbl_sb[:, t:t + 1],
                        in1=xb,
                        op0=mybir.AluOpType.is_equal, op1=mybir.AluOpType.mult,
                        accum_out=G[:, t:t + 1])
                hcols = slice(chunk * TILES_PER_DMA + h * HALF,
                              chunk * TILES_PER_DMA + (h + 1) * HALF)
                if n_a:
                    nc.vector.tensor_scalar_mul(NG[:, hcols], G[:, hcols], -1.0)
                    nc.vector.tensor_copy(GB[:, hcols], G[:, hcols])
                for kk in ks:
                    t = chunk * TILES_PER_DMA + kk
                    if assign[t] == "D":
                        d_tiles.append(kk)
                        continue
                    xb = xw[:, kk * C:(kk + 1) * C]
                    gc = G[:, t:t + 1]
                    u = a_seen % 2
                    if u == 0:
                        rpair = prs.tile([P, 2 * C], BF16, tag="rpair",
                                         name="rpair")
                    rb = rpair[:, u * C:(u + 1) * C]
                    # sign(g - x) = -sign(x - g): bias is the raw G column;
                    # gb^T @ sign(g-x) = -(g^T @ sign(x-g)), fixed in tail.
                    sg = scr.tile([P, C], BF16, tag="sg", name="sg")
                    nc.scalar.activation(
                        sg[:], xb, mybir.ActivationFunctionType.Sign,
                        bias=gc, scale=-1.0)
                    nc.scalar.activation(
                        rb, xb, mybir.ActivationFunctionType.Relu,
                        bias=NG[:, t:t + 1], scale=1.0)
                    nc.tensor.matmul(ps_s[u][:1, :C], GB[:, t:t + 1], sg[:],
                                     start=(mm_s[u] == 0),
                                     stop=(mm_s[u] == n_a // 2 - 1))
                    mm_s[u] += 1
                    if u == 1:
                        nc.tensor.matmul(ps_r[:1, :], ones_bf[:], rpair[:],
                                         start=(mm_r == 0),
                                         stop=(mm_r == n_a // 2 - 1))
                        mm_r += 1
                    a_seen += 1
            for kk in d_tiles:
                t = chunk * TILES_PER_DMA + kk
                xb = xw[:, kk * C:(kk + 1) * C]
                gc = G[:, t:t + 1]
                mp = scr.tile([P, C], F32, tag="mp", name="mp")
                nc.vector.scalar_tensor_tensor(
                    out=mp[:], in0=xb, scalar=gc, in1=xb,
                    op0=mybir.AluOpType.is_gt, op1=mybir.AluOpType.mult,
                    accum_out=M[:, t:t + 1])

        # ---- tail ------------------------------------------------------
        tmp = scr.tile([P, N_TILES], F32, tag="tail", name="tmp")
        nc.vector.tensor_scalar_add(tmp[:], G[:], 0.1)
        inv = scr.tile([P, N_TILES], F32, tag="tail2", name="inv")
        nc.vector.reciprocal(inv[:], tmp[:])
        tot = scr.tile([P, N_TILES], F32, tag="tail3", name="tot")
        nc.vector.tensor_tensor(out=tot[:], in0=inv[:], in1=M[:],
                                op=mybir.AluOpType.add)
        rows = singles.tile([P, 1], F32)
        nc.vector.tensor_reduce(rows[:], tot[:], axis=mybir.AxisListType.X,
                                op=mybir.AluOpType.add)
        if n_a:
            L = len(PATTERN)
            nA = sum(1 for c in PATTERN if c == "A")
            a0 = PATTERN.index("A")
            assert all(c == "A" for c in PATTERN[a0:a0 + nA])
            g_a = G.rearrange("p (u k) -> p u k", k=L)[:, :, a0:a0 + nA]
            rows_ga = singles.tile([P, 1], F32)
            nc.vector.tensor_reduce(rows_ga[:], g_a,
                                    axis=mybir.AxisListType.XY,
                                    op=mybir.AluOpType.add)
            rows2 = singles.tile([P, 1], F32)
            nc.vector.tensor_scalar(out=rows2[:], in0=rows_ga[:],
                                    scalar1=127.5, scalar2=None,
                                    op0=mybir.AluOpType.mult)
            rows3 = singles.tile([P, 1], F32)
            nc.vector.tensor_tensor(out=rows3[:], in0=rows[:], in1=rows2[:],
                                    op=mybir.AluOpType.add)
            rows = rows3

        ps_fin = psum.tile([P, 8], F32, tag="fin")
        nc.tensor.matmul(ps_fin[:1, :1], ones[:], rows[:])

        fin = singles.tile([1, 1], F32)
        nc.vector.tensor_copy(fin[:], ps_fin[:1, :1])
        acc_terms = [fin]
        if n_a:
            # + sum(ps_r) - 0.5*sum(ps_s0 + ps_s1)
            cb = singles.tile([1, 1024], F32)
            nc.vector.tensor_copy(cb[:, 0:512], ps_r[:1, :])
            nc.vector.tensor_copy(cb[:, 512:768], ps_s[0][:1, :C])
            nc.vector.tensor_copy(cb[:, 768:1024], ps_s[1][:1, :C])
            tot1 = singles.tile([1, 1], F32)
            nc.vector.tensor_reduce(tot1[:], cb[:, 0:512],
                                    axis=mybir.AxisListType.X,
                                    op=mybir.AluOpType.add)
            # ps_s carries g*sign(g-x) sums; margin needs -(that)/2
            sc = singles.tile([1, 512], F32)
            nc.vector.tensor_scalar(out=sc[:], in0=cb[:, 512:1024],
                                    scalar1=-0.5, scalar2=None,
                                    op0=mybir.AluOpType.mult)
            tot2 = singles.tile([1, 1], F32)
            nc.vector.tensor_reduce(tot2[:], sc[:],
                                    axis=mybir.AxisListType.X,
                                    op=mybir.AluOpType.add)
            acc_terms += [tot1, tot2]
        res = acc_terms[0]
        for ti, term in enumerate(acc_terms[1:]):
            nxt = singles.tile([1, 1], F32, name=f"sumchain{ti}")
            nc.vector.tensor_tensor(out=nxt[:], in0=res[:], in1=term[:],
                                    op=mybir.AluOpType.add)
            res = nxt
        nc.sync.dma_start(out[:], res[:])
        if gout is not None:
            gcp = singles.tile([P, N_TILES], F32, name="gcp")
            nc.vector.tensor_copy(gcp[:], G[:])
            nc.sync.dma_start(gout[:], gcp[:])

    if SPLIT_WAITS:
        _split_multi_waits(nc)
    _NC_CACHE[key] = nc
    return nc


def _prep_inputs(rna_cell_out, rna_cell_label):
    x = np.ascontiguousarray(np.asarray(rna_cell_out, dtype=np.float32))
    l = np.asarray(rna_cell_label).astype(np.int64)
    assert x.shape == (B, C) and l.shape == (B,)
    in_maps = []
    for i in range(N_CORES):
        xs = x[i * B_LOCAL:(i + 1) * B_LOCAL]
        ls = l[i * B_LOCAL:(i + 1) * B_LOCAL]
        lbl = ls.reshape(P, N_TILES).astype(np.float32)
        in_maps.append({"x": xs, "lbl": np.ascontiguousarray(lbl)})
    return in_maps


def kernel(rna_cell_out, rna_cell_label):
    global LAST_RESULTS
    nc = build_nc()
    in_maps = _prep_inputs(rna_cell_out, rna_cell_label)
    res = run_bass_kernel_spmd(nc, in_maps, list(range(N_CORES)),
                               trace=TRACE, **TRACE_KW)
    LAST_RESULTS = res
    parts = [float(res.results[i]["out"][0, 0]) for i in range(N_CORES)]
    loss = np.float32(np.sum(np.array(parts, dtype=np.float64)) / B)
    return np.array([loss], dtype=np.float32)


# revision 13
# speedup vs baseline: 1.0238x; 1.0238x over previous
"""Trainium2 Bass kernel for the CellLoss problem.

loss = mean_i [ 1/(x[i, l_i] + 0.1) + sum_j x[i,j] * (x[i,j] > x[i, l_i]) ]
with x: [131072, 256] f32, l: [131072] int labels in [0, 256).

Pure data parallel across 8 NeuronCores (16384 rows each). Per core,
row r lives on partition r % 128; tile t is rows [t*128, (t+1)*128).

Key structure (v3):
  gather: two-stage. Stage 1 uses gpsimd.dma_gather to fetch, for every
      row, the 64-float (256 B) window of the row that contains its
      label element (window index r*4 + l//64, int16, so 4 calls of
      4096 windows each). Stage 2 extracts the label element from each
      64-wide window with a DVE stt (iota64 == l%64)*w row-accumulated
      into G — a 64-element pass instead of the 256-element pass a
      direct gather needs (~259 ns vs ~569 ns per tile).
  margin, two engine variants cycled by PATTERN per 16-tile chunk:
   "D": DVE stt, (x is_gt g) mult x with row-sum accumulator into M.
   "A": scalar-engine Relu(x-g) and Sign(g-x) passes writing bf16 tiles;
      the tensor engine accumulates GLOBAL sums in PSUM: ones^T @
      relu-pairs, and gb^T @ sign(g-x) per tile (gb = bf16(g), so the
      matmuls run at bf16 rate, 4x the f32 rate). Using
      sum_i g_i*cnt_i = (-sum g*sign(g-x) + 255*sum g)/2 (sign(0)=0 at
      the label), the margin needs only these global sums.
Tail: inv = 1/(G+0.1); per-row totals + the A-tile 127.5*g correction;
partition sum via ones-matmul; one f32 partial per core; the host sums
the 8 partials and divides by B.

bf16 is used ONLY for relu magnitudes (unbiased rounding, ~1e-6 effect),
the sign values (exact), and the g matmul weights for the count term
(~1e-5 effect); G itself stays exact f32 for the inv term and compares.

This walrus accepts one sync wait per instruction; Tile can emit
several. _split_multi_waits() hoists extras onto Drain carriers.
"""

import numpy as np
from contextlib import ExitStack

import concourse.bass as bass
import concourse.mybir as mybir
import concourse.tile as tile
from concourse.bass_utils import run_bass_kernel_spmd

F32 = mybir.dt.float32
BF16 = mybir.dt.bfloat16
I16 = mybir.dt.int16

B, C = 131072, 256
N_CORES = 8
B_LOCAL = B // N_CORES          # 16384
P = 128
N_TILES = B_LOCAL // P          # 128
TILES_PER_DMA = 16              # [128, 4096] f32 = 2 MiB per DMA
N_CHUNKS = N_TILES // TILES_PER_DMA
W = 64                          # window elements (256 B, dma_gather minimum)
WPR = C // W                    # windows per row = 4
N_GCALLS = 4                    # dma_gather calls (int16 index limit)
ROWS_PER_GCALL = B_LOCAL // N_GCALLS    # 4096
TILES_PER_GCALL = N_TILES // N_GCALLS   # 32

# margin engine per tile within each 16-tile chunk ("D" DVE stt /
# "A" scalar engine + tensor engine); "A" count per chunk must be even
# (pairs share one bf16 relu tile for the ones-matmul)
PATTERN = list("AAAAAAAAAADDDDDD")

_NC_CACHE = {}
LAST_RESULTS = None
SPLIT_WAITS = True   # off for CoreSim (its event loop rejects bare Drains)
TRACE = False
TRACE_KW = {}
DEBUG_G = False      # add a gout output carrying the gathered G tile


def _split_multi_waits(nc):
    for f in nc.m.functions:
        for blk in f.blocks:
            insts = list(blk.instructions)
            out = []
            changed = False
            for inst in insts:
                si = inst.sync_info
                if si is not None and si.on_wait is not None and len(si.on_wait) > 1:
                    waits = list(si.on_wait)
                    for w in waits[:-1]:
                        d = mybir.InstDrain(
                            name=nc.get_next_instruction_name(),
                            ins=[], outs=[], bass_is_fusable=False)
                        d.engine = inst.engine
                        d.sync_info = mybir.SyncInfo(on_wait=[w], on_update=[])
                        out.append(d)
                    inst.sync_info = mybir.SyncInfo(
                        on_wait=[waits[-1]], on_update=list(si.on_update or []))
                    changed = True
                out.append(inst)
            if changed:
                blk.instructions = out


def _assignment():
    assert N_TILES % len(PATTERN) == 0
    return [PATTERN[t % len(PATTERN)] for t in range(N_TILES)]


def build_nc():
    key = (tuple(_assignment()), SPLIT_WAITS, DEBUG_G)
    if key in _NC_CACHE:
        return _NC_CACHE[key]

    assign = _assignment()
    a_tiles = [t for t, c in enumerate(assign) if c == "A"]
    n_a = len(a_tiles)
    for c in range(N_CHUNKS):
        n_ac = sum(1 for t in range(c * TILES_PER_DMA, (c + 1) * TILES_PER_DMA)
                   if assign[t] == "A")
        assert n_ac % 2 == 0, "A count per chunk must be even"

    nc = bass.Bass()
    x = nc.declare_dram_parameter("x", [B_LOCAL, C], F32, isOutput=False)
    lbl = nc.declare_dram_parameter("lbl", [P, N_TILES], F32, isOutput=False)
    out = nc.declare_dram_parameter("out", [1, 1], F32, isOutput=True)
    gout = (nc.declare_dram_parameter("gout", [P, N_TILES], F32, isOutput=True)
            if DEBUG_G else None)

    # row r = p*128 + t  ->  partition p, tile t
    xv = x.rearrange("(p t) c -> p (t c)", p=P, t=N_TILES)

    with tile.TileContext(nc) as tc, ExitStack() as ctx:
        singles = ctx.enter_context(tc.tile_pool(name="singles", bufs=1))
        xpool = ctx.enter_context(tc.tile_pool(name="x", bufs=4))
        scr = ctx.enter_context(tc.tile_pool(name="scr", bufs=4))
        prs = ctx.enter_context(tc.tile_pool(name="prs", bufs=4))
        psum = ctx.enter_context(tc.tile_pool(name="psum", bufs=1, space="PSUM"))

        lbl_sb = singles.tile([P, N_TILES], F32)
        nc.sync.dma_start(lbl_sb[:], lbl[:])

        HMB = TILES_PER_DMA // 2 * C   # half-chunk elements per partition

        def issue_xw(chunk):
            xw = xpool.tile([P, TILES_PER_DMA * C], F32, name="xw")
            base = chunk * TILES_PER_DMA * C
            for h2 in (0, 1):
                nc.sync.dma_start(
                    xw[:, h2 * HMB:(h2 + 1) * HMB],
                    xv[:, base + h2 * HMB:base + (h2 + 1) * HMB])
            return xw

        xw0 = issue_xw(0)

        iota_i = singles.tile([P, C], mybir.dt.int32)
        nc.gpsimd.iota(iota_i[:], pattern=[[1, C]], base=0, channel_multiplier=0)
        iota_f = singles.tile([P, C], F32)
        nc.vector.tensor_copy(iota_f[:], iota_i[:])

        ones = singles.tile([P, 1], F32)
        nc.vector.memset(ones[:], 1.0)
        warm = singles.tile([P, 1], F32)
        nc.scalar.activation(warm[:], ones[:],
                             mybir.ActivationFunctionType.Sign,
                             bias=0.0, scale=1.0)

        G = singles.tile([P, N_TILES], F32)
        M = singles.tile([P, N_TILES], F32)      # D-tile margins; A cols = 0
        if n_a:
            ones_bf = singles.tile([P, 1], BF16)
            nc.vector.memset(ones_bf[:], 1.0)
            nc.vector.memset(M[:], 0.0)
            NG = singles.tile([P, N_TILES], F32)   # -g (f32, ACT bias)
            GB = singles.tile([P, N_TILES], BF16)  # bf16 g (matmul weights)
            ps_r = psum.tile([P, 512], F32, tag="ps_r")
            ps_s = [psum.tile([P, 512], F32, tag=f"ps_s{i}", name=f"ps_s{i}")
                    for i in range(2)]

        mm_r = 0
        mm_s = [0, 0]
        a_seen = 0
        rpair = None
        for chunk in range(N_CHUNKS):
            xw = xw0 if chunk == 0 else issue_xw(chunk)
            HALF = TILES_PER_DMA // 2
            d_tiles = []
            for h in range(2):
                ks = range(h * HALF, (h + 1) * HALF)
                for kk in ks:
                    t = chunk * TILES_PER_DMA + kk
                    xb = xw[:, kk * C:(kk + 1) * C]
                    sel = scr.tile([P, C], F32, tag="sel", name="sel")
                    nc.vector.scalar_tensor_tensor(
                        out=sel[:], in0=iota_f[:], scalar=lbl_sb[:, t:t + 1],
                        in1=xb,
                        op0=mybir.AluOpType.is_equal, op1=mybir.AluOpType.mult,
                        accum_out=G[:, t:t + 1])
                hcols = slice(chunk * TILES_PER_DMA + h * HALF,
                              chunk * TILES_PER_DMA + (h + 1) * HALF)
                if n_a:
                    nc.vector.tensor_scalar_mul(NG[:, hcols], G[:, hcols], -1.0)
                    nc.vector.tensor_copy(GB[:, hcols], G[:, hcols])
                for kk in ks:
                    t = chunk * TILES_PER_DMA + kk
                    if assign[t] == "D":
                        d_tiles.append(kk)
                        continue
                    xb = xw[:, kk * C:(kk + 1) * C]
                    gc = G[:, t:t + 1]
                    u = a_seen % 2
                    if u == 0:
                        rpair = prs.tile([P, 2 * C], BF16, tag="rpair",
                                         name="rpair")
                    rb = rpair[:, u * C:(u + 1) * C]
                    # sign(g - x) = -sign(x - g): bias is the raw G column;
                    # gb^T @ sign(g-x) = -(g^T @ sign(x-g)), fixed in tail.
                    sg = scr.tile([P, C], BF16, tag="sg", name="sg")
                    nc.scalar.activation(
                        sg[:], xb, mybir.ActivationFunctionType.Sign,
                        bias=gc, scale=-1.0)
                    nc.scalar.activation(
                        rb, xb, mybir.ActivationFunctionType.Relu,
                        bias=NG[:, t:t + 1], scale=1.0)
                    nc.tensor.matmul(ps_s[u][:1, :C], GB[:, t:t + 1], sg[:],
                                     start=(mm_s[u] == 0),
                                     stop=(mm_s[u] == n_a // 2 - 1))
                    mm_s[u] += 1
                    if u == 1:
                        nc.tensor.matmul(ps_r[:1, :], ones_bf[:], rpair[:],
                                         start=(mm_r == 0),
                                         stop=(mm_r == n_a // 2 - 1))
                        mm_r += 1
                    a_seen += 1
            for kk in d_tiles:
                t = chunk * TILES_PER_DMA + kk
                xb = xw[:, kk * C:(kk + 1) * C]
                gc = G[:, t:t + 1]
                mp = scr.tile([P, C], F32, tag="mp", name="mp")
                nc.vector.scalar_tensor_tensor(
                    out=mp[:], in0=xb, scalar=gc, in1=xb,
                    op0=mybir.AluOpType.is_gt, op1=mybir.AluOpType.mult,
                    accum_out=M[:, t:t + 1])

        # ---- tail ------------------------------------------------------
        tmp = scr.tile([P, N_TILES], F32, tag="tail", name="tmp")
        nc.vector.tensor_scalar_add(tmp[:], G[:], 0.1)
        inv = scr.tile([P, N_TILES], F32, tag="tail2", name="inv")
        nc.vector.reciprocal(inv[:], tmp[:])
        tot = scr.tile([P, N_TILES], F32, tag="tail3", name="tot")
        nc.vector.tensor_tensor(out=tot[:], in0=inv[:], in1=M[:],
                                op=mybir.AluOpType.add)
        rows = singles.tile([P, 1], F32)
        nc.vector.tensor_reduce(rows[:], tot[:], axis=mybir.AxisListType.X,
                                op=mybir.AluOpType.add)
        if n_a:
            L = len(PATTERN)
            nA = sum(1 for c in PATTERN if c == "A")
            a0 = PATTERN.index("A")
            assert all(c == "A" for c in PATTERN[a0:a0 + nA])
            g_a = G.rearrange("p (u k) -> p u k", k=L)[:, :, a0:a0 + nA]
            rows_ga = singles.tile([P, 1], F32)
            nc.vector.tensor_reduce(rows_ga[:], g_a,
                                    axis=mybir.AxisListType.XY,
                                    op=mybir.AluOpType.add)
            rows2 = singles.tile([P, 1], F32)
            nc.vector.tensor_scalar(out=rows2[:], in0=rows_ga[:],
                                    scalar1=127.5, scalar2=None,
                                    op0=mybir.AluOpType.mult)
            rows3 = singles.tile([P, 1], F32)
            nc.vector.tensor_tensor(out=rows3[:], in0=rows[:], in1=rows2[:],
                                    op=mybir.AluOpType.add)
            rows = rows3

        ps_fin = psum.tile([P, 8], F32, tag="fin")
        nc.tensor.matmul(ps_fin[:1, :1], ones[:], rows[:])

        fin = singles.tile([1, 1], F32)
        nc.vector.tensor_copy(fin[:], ps_fin[:1, :1])
        acc_terms = [fin]
        if n_a:
            # + sum(ps_r) - 0.5*sum(ps_s0 + ps_s1)
            cb = singles.tile([1, 1024], F32)
            nc.vector.tensor_copy(cb[:, 0:512], ps_r[:1, :])
            nc.vector.tensor_copy(cb[:, 512:768], ps_s[0][:1, :C])
            nc.vector.tensor_copy(cb[:, 768:1024], ps_s[1][:1, :C])
            tot1 = singles.tile([1, 1], F32)
            nc.vector.tensor_reduce(tot1[:], cb[:, 0:512],
                                    axis=mybir.AxisListType.X,
                                    op=mybir.AluOpType.add)
            # ps_s carries g*sign(g-x) sums; margin needs -(that)/2
            sc = singles.tile([1, 512], F32)
            nc.vector.tensor_scalar(out=sc[:], in0=cb[:, 512:1024],
                                    scalar1=-0.5, scalar2=None,
                                    op0=mybir.AluOpType.mult)
            tot2 = singles.tile([1, 1], F32)
            nc.vector.tensor_reduce(tot2[:], sc[:],
                                    axis=mybir.AxisListType.X,
                                    op=mybir.AluOpType.add)
            acc_terms += [tot1, tot2]
        res = acc_terms[0]
        for ti, term in enumerate(acc_terms[1:]):
            nxt = singles.tile([1, 1], F32, name=f"sumchain{ti}")
            nc.vector.tensor_tensor(out=nxt[:], in0=res[:], in1=term[:],
                                    op=mybir.AluOpType.add)
            res = nxt
        nc.sync.dma_start(out[:], res[:])
        if gout is not None:
            gcp = singles.tile([P, N_TILES], F32, name="gcp")
            nc.vector.tensor_copy(gcp[:], G[:])
            nc.sync.dma_start(gout[:], gcp[:])

    if SPLIT_WAITS:
        _split_multi_waits(nc)
    _NC_CACHE[key] = nc
    return nc


def _prep_inputs(rna_cell_out, rna_cell_label):
    x = np.ascontiguousarray(np.asarray(rna_cell_out, dtype=np.float32))
    l = np.asarray(rna_cell_label).astype(np.int64)
    assert x.shape == (B, C) and l.shape == (B,)
    in_maps = []
    for i in range(N_CORES):
        xs = x[i * B_LOCAL:(i + 1) * B_LOCAL]
        ls = l[i * B_LOCAL:(i + 1) * B_LOCAL]
        lbl = ls.reshape(P, N_TILES).astype(np.float32)
        in_maps.append({"x": xs, "lbl": np.ascontiguousarray(lbl)})
    return in_maps


def kernel(rna_cell_out, rna_cell_label):
    global LAST_RESULTS
    nc = build_nc()
    in_maps = _prep_inputs(rna_cell_out, rna_cell_label)
    res = run_bass_kernel_spmd(nc, in_maps, list(range(N_CORES)),
                               trace=TRACE, **TRACE_KW)
    LAST_RESULTS = res
    parts = [float(res.results[i]["out"][0, 0]) for i in range(N_CORES)]
    loss = np.float32(np.sum(np.array(parts, dtype=np.float64)) / B)
    return np.array([loss], dtype=np.float32)


# revision 14
# speedup vs baseline: 1.0249x; 1.0010x over previous
"""Trainium2 Bass kernel for the CellLoss problem.

loss = mean_i [ 1/(x[i, l_i] + 0.1) + sum_j x[i,j] * (x[i,j] > x[i, l_i]) ]
with x: [131072, 256] f32, l: [131072] int labels in [0, 256).

Pure data parallel across 8 NeuronCores (16384 rows each). Per core,
partition p owns rows [p*128, (p+1)*128); tile t is the [128, 256]
block of row p*128+t per partition.

Structure (per 16-tile chunk, streamed in two 1 MiB DMA halves):
  gather (DVE): per tile one fused stt (iota==l)*x with a per-row sum
      accumulator into G. Emitted in 8-tile half-batches so the scalar
      engine can start each chunk's work after half the gathers.
  margin, two engine variants cycled by PATTERN per 16-tile chunk:
   "D": DVE stt, (x is_gt g) mult x with row-sum accumulator into M.
   "A": scalar-engine Relu(x-g) and Sign(g-x) passes writing bf16 tiles;
      the tensor engine accumulates GLOBAL sums in PSUM: ones^T @
      relu-pairs, and gb^T @ sign(g-x) per tile (gb = bf16(g), so the
      matmuls run at bf16 rate, 4x the f32 rate). Using
      sum_i g_i*cnt_i = (-sum g*sign(g-x) + 255*sum g)/2 (sign(0)=0 at
      the label), the margin needs only these global sums.
Tail: inv = 1/(G+0.1); per-row totals + the A-tile 127.5*g correction;
partition sum via ones-matmul; one f32 partial per core; the host sums
the 8 partials and divides by B.

bf16 is used ONLY for relu magnitudes (unbiased rounding, ~1e-6 effect),
the sign values (exact), and the g matmul weights for the count term
(~1e-5 effect); G itself stays exact f32 for the inv term and compares.
A-tiles lead each chunk (pattern head) and NG/GB bias/weight prep is
batched per half-chunk, keeping the scalar engine fed while DVE runs
the gathers; a warm-up Sign activation pulls the 1.3 us activation
table load into the startup DMA window.

This walrus accepts one sync wait per instruction; Tile can emit
several. _split_multi_waits() hoists extras onto Drain carriers.
"""

import numpy as np
from contextlib import ExitStack

import concourse.bass as bass
import concourse.mybir as mybir
import concourse.tile as tile
from concourse.bass_utils import run_bass_kernel_spmd

F32 = mybir.dt.float32
BF16 = mybir.dt.bfloat16

B, C = 131072, 256
N_CORES = 8
B_LOCAL = B // N_CORES          # 16384
P = 128
N_TILES = B_LOCAL // P          # 128
TILES_PER_DMA = 16              # [128, 4096] f32 = 2 MiB per DMA
N_CHUNKS = N_TILES // TILES_PER_DMA

# margin engine per tile within each 16-tile chunk ("D" DVE stt /
# "A" scalar engine + tensor engine); "A" count per chunk must be even
# (pairs share one bf16 relu tile for the ones-matmul)
PATTERN = list("AAAAAAAAAADDDDDD")

_NC_CACHE = {}
LAST_RESULTS = None
SPLIT_WAITS = True   # off for CoreSim (its event loop rejects bare Drains)
TRACE = False
TRACE_KW = {}
DEBUG_G = False      # add a gout output carrying the gathered G tile


def _split_multi_waits(nc):
    for f in nc.m.functions:
        for blk in f.blocks:
            insts = list(blk.instructions)
            out = []
            changed = False
            for inst in insts:
                si = inst.sync_info
                if si is not None and si.on_wait is not None and len(si.on_wait) > 1:
                    waits = list(si.on_wait)
                    for w in waits[:-1]:
                        d = mybir.InstDrain(
                            name=nc.get_next_instruction_name(),
                            ins=[], outs=[], bass_is_fusable=False)
                        d.engine = inst.engine
                        d.sync_info = mybir.SyncInfo(on_wait=[w], on_update=[])
                        out.append(d)
                    inst.sync_info = mybir.SyncInfo(
                        on_wait=[waits[-1]], on_update=list(si.on_update or []))
                    changed = True
                out.append(inst)
            if changed:
                blk.instructions = out


def _assignment():
    assert N_TILES % len(PATTERN) == 0
    return [PATTERN[t % len(PATTERN)] for t in range(N_TILES)]


def build_nc():
    key = (tuple(_assignment()), SPLIT_WAITS, DEBUG_G)
    if key in _NC_CACHE:
        return _NC_CACHE[key]

    assign = _assignment()
    a_tiles = [t for t, c in enumerate(assign) if c == "A"]
    n_a = len(a_tiles)
    for c in range(N_CHUNKS):
        n_ac = sum(1 for t in range(c * TILES_PER_DMA, (c + 1) * TILES_PER_DMA)
                   if assign[t] == "A")
        assert n_ac % 2 == 0, "A count per chunk must be even"

    nc = bass.Bass()
    x = nc.declare_dram_parameter("x", [B_LOCAL, C], F32, isOutput=False)
    lbl = nc.declare_dram_parameter("lbl", [P, N_TILES], F32, isOutput=False)
    out = nc.declare_dram_parameter("out", [1, 1], F32, isOutput=True)
    gout = (nc.declare_dram_parameter("gout", [P, N_TILES], F32, isOutput=True)
            if DEBUG_G else None)

    # row r = p*128 + t  ->  partition p, tile t
    xv = x.rearrange("(p t) c -> p (t c)", p=P, t=N_TILES)

    with tile.TileContext(nc) as tc, ExitStack() as ctx:
        singles = ctx.enter_context(tc.tile_pool(name="singles", bufs=1))
        xpool = ctx.enter_context(tc.tile_pool(name="x", bufs=4))
        scr = ctx.enter_context(tc.tile_pool(name="scr", bufs=4))
        prs = ctx.enter_context(tc.tile_pool(name="prs", bufs=4))
        psum = ctx.enter_context(tc.tile_pool(name="psum", bufs=1, space="PSUM"))

        lbl_sb = singles.tile([P, N_TILES], F32)
        nc.sync.dma_start(lbl_sb[:], lbl[:])

        HMB = TILES_PER_DMA // 2 * C   # half-chunk elements per partition

        def issue_xw(chunk):
            xw = xpool.tile([P, TILES_PER_DMA * C], F32, name="xw")
            base = chunk * TILES_PER_DMA * C
            for h2 in (0, 1):
                nc.sync.dma_start(
                    xw[:, h2 * HMB:(h2 + 1) * HMB],
                    xv[:, base + h2 * HMB:base + (h2 + 1) * HMB])
            return xw

        xw0 = issue_xw(0)

        iota_i = singles.tile([P, C], mybir.dt.int32)
        nc.gpsimd.iota(iota_i[:], pattern=[[1, C]], base=0, channel_multiplier=0)
        iota_f = singles.tile([P, C], F32)
        nc.vector.tensor_copy(iota_f[:], iota_i[:])

        ones = singles.tile([P, 1], F32)
        nc.vector.memset(ones[:], 1.0)
        warm = singles.tile([P, 1], F32)
        nc.scalar.activation(warm[:], ones[:],
                             mybir.ActivationFunctionType.Sign,
                             bias=0.0, scale=1.0)

        G = singles.tile([P, N_TILES], F32)
        M = singles.tile([P, N_TILES], F32)      # D-tile margins; A cols = 0
        if n_a:
            ones_bf = singles.tile([P, 1], BF16)
            nc.vector.memset(ones_bf[:], 1.0)
            nc.vector.memset(M[:], 0.0)
            NG = singles.tile([P, N_TILES], F32)   # -g (f32, ACT bias)
            GB = singles.tile([P, N_TILES], BF16)  # bf16 g (matmul weights)
            ps_r = psum.tile([P, 512], F32, tag="ps_r")
            ps_s = [psum.tile([P, 512], F32, tag=f"ps_s{i}", name=f"ps_s{i}")
                    for i in range(2)]

        mm_r = 0
        mm_s = [0, 0]
        a_seen = 0
        rpair = None
        for chunk in range(N_CHUNKS):
            xw = xw0 if chunk == 0 else issue_xw(chunk)
            HALF = TILES_PER_DMA // 2
            d_tiles = []
            for h in range(2):
                ks = range(h * HALF, (h + 1) * HALF)
                for kk in ks:
                    t = chunk * TILES_PER_DMA + kk
                    xb = xw[:, kk * C:(kk + 1) * C]
                    sel = scr.tile([P, C], F32, tag="sel", name="sel")
                    nc.vector.scalar_tensor_tensor(
                        out=sel[:], in0=iota_f[:], scalar=lbl_sb[:, t:t + 1],
                        in1=xb,
                        op0=mybir.AluOpType.is_equal, op1=mybir.AluOpType.mult,
                        accum_out=G[:, t:t + 1])
                hcols = slice(chunk * TILES_PER_DMA + h * HALF,
                              chunk * TILES_PER_DMA + (h + 1) * HALF)
                if n_a:
                    nc.vector.tensor_scalar_mul(NG[:, hcols], G[:, hcols], -1.0)
                    nc.vector.tensor_copy(GB[:, hcols], G[:, hcols])
                for kk in ks:
                    t = chunk * TILES_PER_DMA + kk
                    if assign[t] == "D":
                        d_tiles.append(kk)
                        continue
                    xb = xw[:, kk * C:(kk + 1) * C]
                    gc = G[:, t:t + 1]
                    u = a_seen % 2
                    if u == 0:
                        rpair = prs.tile([P, 2 * C], BF16, tag="rpair",
                                         name="rpair")
                    rb = rpair[:, u * C:(u + 1) * C]
                    # sign(g - x) = -sign(x - g): bias is the raw G column;
                    # gb^T @ sign(g-x) = -(g^T @ sign(x-g)), fixed in tail.
                    sg = scr.tile([P, C], BF16, tag="sg", name="sg")
                    nc.scalar.activation(
                        sg[:], xb, mybir.ActivationFunctionType.Sign,
                        bias=gc, scale=-1.0)
                    nc.scalar.activation(
                        rb, xb, mybir.ActivationFunctionType.Relu,
                        bias=NG[:, t:t + 1], scale=1.0)
                    nc.tensor.matmul(ps_s[u][:1, :C], GB[:, t:t + 1], sg[:],
                                     start=(mm_s[u] == 0),
                                     stop=(mm_s[u] == n_a // 2 - 1))
                    mm_s[u] += 1
                    if u == 1:
                        nc.tensor.matmul(ps_r[:1, :], ones_bf[:], rpair[:],
                                         start=(mm_r == 0),
                                         stop=(mm_r == n_a // 2 - 1))
                        mm_r += 1
                    a_seen += 1
            for kk in d_tiles:
                t = chunk * TILES_PER_DMA + kk
                xb = xw[:, kk * C:(kk + 1) * C]
                gc = G[:, t:t + 1]
                mp = scr.tile([P, C], F32, tag="mp", name="mp")
                nc.vector.scalar_tensor_tensor(
                    out=mp[:], in0=xb, scalar=gc, in1=xb,
                    op0=mybir.AluOpType.is_gt, op1=mybir.AluOpType.mult,
                    accum_out=M[:, t:t + 1])

        # ---- tail ------------------------------------------------------
        tmp = scr.tile([P, N_TILES], F32, tag="tail", name="tmp")
        nc.vector.tensor_scalar_add(tmp[:], G[:], 0.1)
        inv = scr.tile([P, N_TILES], F32, tag="tail2", name="inv")
        nc.vector.reciprocal(inv[:], tmp[:])
        tot = scr.tile([P, N_TILES], F32, tag="tail3", name="tot")
        nc.vector.tensor_tensor(out=tot[:], in0=inv[:], in1=M[:],
                                op=mybir.AluOpType.add)
        rows = singles.tile([P, 1], F32)
        nc.vector.tensor_reduce(rows[:], tot[:], axis=mybir.AxisListType.X,
                                op=mybir.AluOpType.add)
        if n_a:
            L = len(PATTERN)
            nA = sum(1 for c in PATTERN if c == "A")
            a0 = PATTERN.index("A")
            assert all(c == "A" for c in PATTERN[a0:a0 + nA])
            g_a = G.rearrange("p (u k) -> p u k", k=L)[:, :, a0:a0 + nA]
            rows_ga = singles.tile([P, 1], F32)
            nc.vector.tensor_reduce(rows_ga[:], g_a,
                                    axis=mybir.AxisListType.XY,
                                    op=mybir.AluOpType.add)
            rows2 = singles.tile([P, 1], F32)
            nc.vector.tensor_scalar(out=rows2[:], in0=rows_ga[:],
                                    scalar1=127.5, scalar2=None,
                                    op0=mybir.AluOpType.mult)
            rows3 = singles.tile([P, 1], F32)
            nc.vector.tensor_tensor(out=rows3[:], in0=rows[:], in1=rows2[:],
                                    op=mybir.AluOpType.add)
            rows = rows3

        ps_fin = psum.tile([P, 8], F32, tag="fin")
        nc.tensor.matmul(ps_fin[:1, :1], ones[:], rows[:])

        fin = singles.tile([1, 1], F32)
        nc.vector.tensor_copy(fin[:], ps_fin[:1, :1])
        acc_terms = [fin]
        if n_a:
            # + sum(ps_r) - 0.5*sum(ps_s0 + ps_s1)
            cb = singles.tile([1, 1024], F32)
            nc.vector.tensor_copy(cb[:, 0:512], ps_r[:1, :])
            nc.vector.tensor_copy(cb[:, 512:768], ps_s[0][:1, :C])
            nc.vector.tensor_copy(cb[:, 768:1024], ps_s[1][:1, :C])
            tot1 = singles.tile([1, 1], F32)
            nc.vector.tensor_reduce(tot1[:], cb[:, 0:512],
                                    axis=mybir.AxisListType.X,
                                    op=mybir.AluOpType.add)
            # ps_s carries g*sign(g-x) sums; margin needs -(that)/2
            sc = singles.tile([1, 512], F32)
            nc.vector.tensor_scalar(out=sc[:], in0=cb[:, 512:1024],
                                    scalar1=-0.5, scalar2=None,
                                    op0=mybir.AluOpType.mult)
            tot2 = singles.tile([1, 1], F32)
            nc.vector.tensor_reduce(tot2[:], sc[:],
                                    axis=mybir.AxisListType.X,
                                    op=mybir.AluOpType.add)
            acc_terms += [tot1, tot2]
        res = acc_terms[0]
        for ti, term in enumerate(acc_terms[1:]):
            nxt = singles.tile([1, 1], F32, name=f"sumchain{ti}")
            nc.vector.tensor_tensor(out=nxt[:], in0=res[:], in1=term[:],
                                    op=mybir.AluOpType.add)
            res = nxt
        nc.sync.dma_start(out[:], res[:])
        if gout is not None:
            gcp = singles.tile([P, N_TILES], F32, name="gcp")
            nc.vector.tensor_copy(gcp[:], G[:])
            nc.sync.dma_start(gout[:], gcp[:])

    if SPLIT_WAITS:
        _split_multi_waits(nc)
    _NC_CACHE[key] = nc
    return nc


def _prep_inputs(rna_cell_out, rna_cell_label):
    x = np.ascontiguousarray(np.asarray(rna_cell_out, dtype=np.float32))
    l = np.asarray(rna_cell_label).astype(np.int64)
    assert x.shape == (B, C) and l.shape == (B,)
    in_maps = []
    for i in range(N_CORES):
        xs = x[i * B_LOCAL:(i + 1) * B_LOCAL]
        ls = l[i * B_LOCAL:(i + 1) * B_LOCAL]
        lbl = ls.reshape(P, N_TILES).astype(np.float32)
        in_maps.append({"x": xs, "lbl": np.ascontiguousarray(lbl)})
    return in_maps


def kernel(rna_cell_out, rna_cell_label):
    global LAST_RESULTS
    nc = build_nc()
    in_maps = _prep_inputs(rna_cell_out, rna_cell_label)
    res = run_bass_kernel_spmd(nc, in_maps, list(range(N_CORES)),
                               trace=TRACE, **TRACE_KW)
    LAST_RESULTS = res
    parts = [float(res.results[i]["out"][0, 0]) for i in range(N_CORES)]
    loss = np.float32(np.sum(np.array(parts, dtype=np.float64)) / B)
    return np.array([loss], dtype=np.float32)


# revision 15
# speedup vs baseline: 1.0422x; 1.0169x over previous
"""Trainium2 Bass kernel for the CellLoss problem.

loss = mean_i [ 1/(x[i, l_i] + 0.1) + sum_j x[i,j] * (x[i,j] > x[i, l_i]) ]
with x: [131072, 256] f32, l: [131072] int labels in [0, 256).

Pure data parallel across 8 NeuronCores (16384 rows each). Per core,
partition p owns rows [p*128, (p+1)*128); tile t is the [128, 256]
block of row p*128+t per partition.

Structure (per 16-tile chunk, streamed in two 1 MiB DMA halves):
  gather (DVE): per tile one fused stt (iota==l)*x with a per-row sum
      accumulator into G. Emitted in 8-tile half-batches so the scalar
      engine can start each chunk's work after half the gathers.
  margin, two engine variants cycled by PATTERN per 16-tile chunk:
   "D": DVE stt, (x is_gt g) mult x with row-sum accumulator into M.
   "A": scalar-engine Relu(x-g) and Sign(g-x) passes writing bf16 tiles;
      the tensor engine accumulates GLOBAL sums in PSUM: ones^T @
      relu-pairs, and gb^T @ sign(g-x) per tile (gb = bf16(g), so the
      matmuls run at bf16 rate, 4x the f32 rate). Using
      sum_i g_i*cnt_i = (-sum g*sign(g-x) + 255*sum g)/2 (sign(0)=0 at
      the label), the margin needs only these global sums.
Tail: inv = 1/(G+0.1); per-row totals + the A-tile 127.5*g correction;
partition sum via ones-matmul; one f32 partial per core; the host sums
the 8 partials and divides by B.

bf16 is used ONLY for relu magnitudes (unbiased rounding, ~1e-6 effect),
the sign values (exact), and the g matmul weights for the count term
(~1e-5 effect); G itself stays exact f32 for the inv term and compares.
A-tiles lead each chunk (pattern head) and NG/GB bias/weight prep is
batched per half-chunk, keeping the scalar engine fed while DVE runs
the gathers; a warm-up Sign activation pulls the 1.3 us activation
table load into the startup DMA window.

This walrus accepts one sync wait per instruction; Tile can emit
several. _split_multi_waits() hoists extras onto Drain carriers.
"""

import numpy as np
from contextlib import ExitStack

import concourse.bass as bass
import concourse.mybir as mybir
import concourse.tile as tile
from concourse.bass_utils import run_bass_kernel_spmd

F32 = mybir.dt.float32
BF16 = mybir.dt.bfloat16

B, C = 131072, 256
N_CORES = 8
B_LOCAL = B // N_CORES          # 16384
P = 128
N_TILES = B_LOCAL // P          # 128
TILES_PER_DMA = 16              # [128, 4096] f32 = 2 MiB per DMA
N_CHUNKS = N_TILES // TILES_PER_DMA

# margin engine per tile within each 16-tile chunk ("D" DVE stt /
# "A" scalar engine + tensor engine); "A" count per chunk must be even
# (pairs share one bf16 relu tile for the ones-matmul)
PATTERN = list("AAAAAAAAAADDDDDD")

_NC_CACHE = {}
LAST_RESULTS = None
SPLIT_WAITS = True   # off for CoreSim (its event loop rejects bare Drains)
TRACE = False
TRACE_KW = {}
DEBUG_G = False      # add a gout output carrying the gathered G tile


def _split_multi_waits(nc):
    for f in nc.m.functions:
        for blk in f.blocks:
            insts = list(blk.instructions)
            out = []
            changed = False
            for inst in insts:
                si = inst.sync_info
                if si is not None and si.on_wait is not None and len(si.on_wait) > 1:
                    waits = list(si.on_wait)
                    for w in waits[:-1]:
                        d = mybir.InstDrain(
                            name=nc.get_next_instruction_name(),
                            ins=[], outs=[], bass_is_fusable=False)
                        d.engine = inst.engine
                        d.sync_info = mybir.SyncInfo(on_wait=[w], on_update=[])
                        out.append(d)
                    inst.sync_info = mybir.SyncInfo(
                        on_wait=[waits[-1]], on_update=list(si.on_update or []))
                    changed = True
                out.append(inst)
            if changed:
                blk.instructions = out


def _assignment():
    assert N_TILES % len(PATTERN) == 0
    return [PATTERN[t % len(PATTERN)] for t in range(N_TILES)]


def build_nc():
    key = (tuple(_assignment()), SPLIT_WAITS, DEBUG_G)
    if key in _NC_CACHE:
        return _NC_CACHE[key]

    assign = _assignment()
    a_tiles = [t for t, c in enumerate(assign) if c == "A"]
    n_a = len(a_tiles)
    for c in range(N_CHUNKS):
        n_ac = sum(1 for t in range(c * TILES_PER_DMA, (c + 1) * TILES_PER_DMA)
                   if assign[t] == "A")
        assert n_ac % 2 == 0, "A count per chunk must be even"

    nc = bass.Bass()
    x = nc.declare_dram_parameter("x", [B_LOCAL, C], F32, isOutput=False)
    lbl = nc.declare_dram_parameter("lbl", [P, N_TILES], F32, isOutput=False)
    out = nc.declare_dram_parameter("out", [1, 1], F32, isOutput=True)
    gout = (nc.declare_dram_parameter("gout", [P, N_TILES], F32, isOutput=True)
            if DEBUG_G else None)

    # row r = p*128 + t  ->  partition p, tile t
    xv = x.rearrange("(p t) c -> p (t c)", p=P, t=N_TILES)

    with tile.TileContext(nc) as tc, ExitStack() as ctx:
        singles = ctx.enter_context(tc.tile_pool(name="singles", bufs=1))
        xpool = ctx.enter_context(tc.tile_pool(name="x", bufs=4))
        scr = ctx.enter_context(tc.tile_pool(name="scr", bufs=4))
        prs = ctx.enter_context(tc.tile_pool(name="prs", bufs=4))
        psum = ctx.enter_context(tc.tile_pool(name="psum", bufs=1, space="PSUM"))

        lbl_sb = singles.tile([P, N_TILES], F32)
        nc.sync.dma_start(lbl_sb[:], lbl[:])

        HMB = TILES_PER_DMA // 2 * C   # half-chunk elements per partition

        def issue_xw(chunk, splits=2):
            xw = xpool.tile([P, TILES_PER_DMA * C], F32, name="xw")
            base = chunk * TILES_PER_DMA * C
            step = TILES_PER_DMA * C // splits
            for h2 in range(splits):
                nc.sync.dma_start(
                    xw[:, h2 * step:(h2 + 1) * step],
                    xv[:, base + h2 * step:base + (h2 + 1) * step])
            return xw

        xw0 = issue_xw(0, splits=4)

        iota_i = singles.tile([P, C], mybir.dt.int32)
        nc.gpsimd.iota(iota_i[:], pattern=[[1, C]], base=0, channel_multiplier=0)
        iota_f = singles.tile([P, C], F32)
        nc.vector.tensor_copy(iota_f[:], iota_i[:])

        ones = singles.tile([P, 1], F32)
        nc.vector.memset(ones[:], 1.0)
        warm = singles.tile([P, 1], F32)
        nc.scalar.activation(warm[:], ones[:],
                             mybir.ActivationFunctionType.Sign,
                             bias=0.0, scale=1.0)

        G = singles.tile([P, N_TILES], F32)
        M = singles.tile([P, N_TILES], F32)      # D-tile margins; A cols = 0
        if n_a:
            ones_bf = singles.tile([P, 1], BF16)
            nc.vector.memset(ones_bf[:], 1.0)
            nc.vector.memset(M[:], 0.0)
            NGB = singles.tile([P, N_TILES], BF16)  # -g: ACT bias + weights
            ps_r = psum.tile([P, 512], F32, tag="ps_r")
            ps_s = [psum.tile([P, 512], F32, tag=f"ps_s{i}", name=f"ps_s{i}")
                    for i in range(2)]

        mm_r = 0
        mm_s = [0, 0]
        a_seen = 0
        rpair = None
        for chunk in range(N_CHUNKS):
            xw = xw0 if chunk == 0 else issue_xw(chunk)
            HALF = TILES_PER_DMA // 2
            d_tiles = []
            for h in range(2):
                ks = range(h * HALF, (h + 1) * HALF)
                for kk in ks:
                    t = chunk * TILES_PER_DMA + kk
                    xb = xw[:, kk * C:(kk + 1) * C]
                    sel = scr.tile([P, C], F32, tag="sel", name="sel")
                    nc.vector.scalar_tensor_tensor(
                        out=sel[:], in0=iota_f[:], scalar=lbl_sb[:, t:t + 1],
                        in1=xb,
                        op0=mybir.AluOpType.is_equal, op1=mybir.AluOpType.mult,
                        accum_out=G[:, t:t + 1])
                hcols = slice(chunk * TILES_PER_DMA + h * HALF,
                              chunk * TILES_PER_DMA + (h + 1) * HALF)
                if n_a:
                    nc.vector.tensor_scalar_mul(NGB[:, hcols], G[:, hcols], -1.0)
                for kk in ks:
                    t = chunk * TILES_PER_DMA + kk
                    if assign[t] == "D":
                        d_tiles.append(kk)
                        continue
                    xb = xw[:, kk * C:(kk + 1) * C]
                    gc = G[:, t:t + 1]
                    u = a_seen % 2
                    if u == 0:
                        rpair = prs.tile([P, 2 * C], BF16, tag="rpair",
                                         name="rpair")
                    rb = rpair[:, u * C:(u + 1) * C]
                    # sign(g - x) = -sign(x - g): bias is the raw G column;
                    # (-g)^T @ sign(g-x) = g^T @ sign(x-g), +0.5 in tail.
                    sg = scr.tile([P, C], BF16, tag="sg", name="sg")
                    nc.scalar.activation(
                        sg[:], xb, mybir.ActivationFunctionType.Sign,
                        bias=gc, scale=-1.0)
                    nc.scalar.activation(
                        rb, xb, mybir.ActivationFunctionType.Relu,
                        bias=NGB[:, t:t + 1], scale=1.0)
                    nc.tensor.matmul(ps_s[u][:1, :C], NGB[:, t:t + 1], sg[:],
                                     start=(mm_s[u] == 0),
                                     stop=(mm_s[u] == n_a // 2 - 1))
                    mm_s[u] += 1
                    if u == 1:
                        nc.tensor.matmul(ps_r[:1, :], ones_bf[:], rpair[:],
                                         start=(mm_r == 0),
                                         stop=(mm_r == n_a // 2 - 1))
                        mm_r += 1
                    a_seen += 1
            for kk in d_tiles:
                t = chunk * TILES_PER_DMA + kk
                xb = xw[:, kk * C:(kk + 1) * C]
                gc = G[:, t:t + 1]
                mp = scr.tile([P, C], F32, tag="mp", name="mp")
                nc.vector.scalar_tensor_tensor(
                    out=mp[:], in0=xb, scalar=gc, in1=xb,
                    op0=mybir.AluOpType.is_gt, op1=mybir.AluOpType.mult,
                    accum_out=M[:, t:t + 1])

        # ---- tail ------------------------------------------------------
        tmp = scr.tile([P, N_TILES], F32, tag="tail", name="tmp")
        nc.vector.tensor_scalar_add(tmp[:], G[:], 0.1)
        inv = scr.tile([P, N_TILES], F32, tag="tail2", name="inv")
        nc.vector.reciprocal(inv[:], tmp[:])
        tot = scr.tile([P, N_TILES], F32, tag="tail3", name="tot")
        nc.vector.tensor_tensor(out=tot[:], in0=inv[:], in1=M[:],
                                op=mybir.AluOpType.add)
        rows = singles.tile([P, 1], F32)
        nc.vector.tensor_reduce(rows[:], tot[:], axis=mybir.AxisListType.X,
                                op=mybir.AluOpType.add)
        if n_a:
            L = len(PATTERN)
            nA = sum(1 for c in PATTERN if c == "A")
            a0 = PATTERN.index("A")
            assert all(c == "A" for c in PATTERN[a0:a0 + nA])
            g_a = G.rearrange("p (u k) -> p u k", k=L)[:, :, a0:a0 + nA]
            rows_ga = singles.tile([P, 1], F32)
            nc.vector.tensor_reduce(rows_ga[:], g_a,
                                    axis=mybir.AxisListType.XY,
                                    op=mybir.AluOpType.add)
            rows2 = singles.tile([P, 1], F32)
            nc.vector.tensor_scalar(out=rows2[:], in0=rows_ga[:],
                                    scalar1=127.5, scalar2=None,
                                    op0=mybir.AluOpType.mult)
            rows3 = singles.tile([P, 1], F32)
            nc.vector.tensor_tensor(out=rows3[:], in0=rows[:], in1=rows2[:],
                                    op=mybir.AluOpType.add)
            rows = rows3

        ps_fin = psum.tile([P, 8], F32, tag="fin")
        nc.tensor.matmul(ps_fin[:1, :1], ones[:], rows[:])

        fin = singles.tile([1, 1], F32)
        nc.vector.tensor_copy(fin[:], ps_fin[:1, :1])
        acc_terms = [fin]
        if n_a:
            # + sum(ps_r) - 0.5*sum(ps_s0 + ps_s1)
            cb = singles.tile([1, 1024], F32)
            nc.vector.tensor_copy(cb[:, 0:512], ps_r[:1, :])
            nc.vector.tensor_copy(cb[:, 512:768], ps_s[0][:1, :C])
            nc.vector.tensor_copy(cb[:, 768:1024], ps_s[1][:1, :C])
            tot1 = singles.tile([1, 1], F32)
            nc.vector.tensor_reduce(tot1[:], cb[:, 0:512],
                                    axis=mybir.AxisListType.X,
                                    op=mybir.AluOpType.add)
            # ps_s carries (-g)*sign(g-x) = g*sign(x-g) sums; margin
            # needs +(that)/2
            sc = singles.tile([1, 512], F32)
            nc.vector.tensor_scalar(out=sc[:], in0=cb[:, 512:1024],
                                    scalar1=0.5, scalar2=None,
                                    op0=mybir.AluOpType.mult)
            tot2 = singles.tile([1, 1], F32)
            nc.vector.tensor_reduce(tot2[:], sc[:],
                                    axis=mybir.AxisListType.X,
                                    op=mybir.AluOpType.add)
            acc_terms += [tot1, tot2]
        res = acc_terms[0]
        for ti, term in enumerate(acc_terms[1:]):
            nxt = singles.tile([1, 1], F32, name=f"sumchain{ti}")
            nc.vector.tensor_tensor(out=nxt[:], in0=res[:], in1=term[:],
                                    op=mybir.AluOpType.add)
            res = nxt
        nc.sync.dma_start(out[:], res[:])
        if gout is not None:
            gcp = singles.tile([P, N_TILES], F32, name="gcp")
            nc.vector.tensor_copy(gcp[:], G[:])
            nc.sync.dma_start(gout[:], gcp[:])

    if SPLIT_WAITS:
        _split_multi_waits(nc)
    _NC_CACHE[key] = nc
    return nc


def _prep_inputs(rna_cell_out, rna_cell_label):
    x = np.ascontiguousarray(np.asarray(rna_cell_out, dtype=np.float32))
    l = np.asarray(rna_cell_label).astype(np.int64)
    assert x.shape == (B, C) and l.shape == (B,)
    in_maps = []
    for i in range(N_CORES):
        xs = x[i * B_LOCAL:(i + 1) * B_LOCAL]
        ls = l[i * B_LOCAL:(i + 1) * B_LOCAL]
        lbl = ls.reshape(P, N_TILES).astype(np.float32)
        in_maps.append({"x": xs, "lbl": np.ascontiguousarray(lbl)})
    return in_maps


def kernel(rna_cell_out, rna_cell_label):
    global LAST_RESULTS
    nc = build_nc()
    in_maps = _prep_inputs(rna_cell_out, rna_cell_label)
    res = run_bass_kernel_spmd(nc, in_maps, list(range(N_CORES)),
                               trace=TRACE, **TRACE_KW)
    LAST_RESULTS = res
    parts = [float(res.results[i]["out"][0, 0]) for i in range(N_CORES)]
    loss = np.float32(np.sum(np.array(parts, dtype=np.float64)) / B)
    return np.array([loss], dtype=np.float32)


# revision 16
# speedup vs baseline: 1.0583x; 1.0155x over previous
"""Trainium2 Bass kernel for the CellLoss problem.

loss = mean_i [ 1/(x[i, l_i] + 0.1) + sum_j x[i,j] * (x[i,j] > x[i, l_i]) ]
with x: [131072, 256] f32, l: [131072] int labels in [0, 256).

Pure data parallel across 8 NeuronCores (16384 rows each). Per core,
partition p owns rows [p*128, (p+1)*128); tile t is the [128, 256]
block of row p*128+t per partition.

Structure (per 16-tile chunk, streamed in two 1 MiB DMA halves):
  gather (DVE): per tile one fused stt (iota==l)*x with a per-row sum
      accumulator into G. Emitted in 8-tile half-batches so the scalar
      engine can start each chunk's work after half the gathers.
  margin, two engine variants cycled by PATTERN per 16-tile chunk:
   "D": DVE stt, (x is_gt g) mult x with row-sum accumulator into M.
   "A": scalar-engine Relu(x-g) and Sign(g-x) passes writing bf16 tiles;
      the tensor engine accumulates GLOBAL sums in PSUM: ones^T @
      relu-pairs, and gb^T @ sign(g-x) per tile (gb = bf16(g), so the
      matmuls run at bf16 rate, 4x the f32 rate). Using
      sum_i g_i*cnt_i = (-sum g*sign(g-x) + 255*sum g)/2 (sign(0)=0 at
      the label), the margin needs only these global sums.
Tail: inv = 1/(G+0.1); per-row totals + the A-tile 127.5*g correction;
partition sum via ones-matmul; one f32 partial per core; the host sums
the 8 partials and divides by B.

bf16 is used ONLY for relu magnitudes (unbiased rounding, ~1e-6 effect),
the sign values (exact), and the g matmul weights for the count term
(~1e-5 effect); G itself stays exact f32 for the inv term and compares.
A-tiles lead each chunk (pattern head) and NG/GB bias/weight prep is
batched per half-chunk, keeping the scalar engine fed while DVE runs
the gathers; a warm-up Sign activation pulls the 1.3 us activation
table load into the startup DMA window.

This walrus accepts one sync wait per instruction; Tile can emit
several. _split_multi_waits() hoists extras onto Drain carriers.
"""

import numpy as np
from contextlib import ExitStack

import concourse.bass as bass
import concourse.mybir as mybir
import concourse.tile as tile
from concourse.bass_utils import run_bass_kernel_spmd

F32 = mybir.dt.float32
BF16 = mybir.dt.bfloat16

B, C = 131072, 256
N_CORES = 8
B_LOCAL = B // N_CORES          # 16384
P = 128
N_TILES = B_LOCAL // P          # 128
TILES_PER_DMA = 16              # [128, 4096] f32 = 2 MiB per DMA
N_CHUNKS = N_TILES // TILES_PER_DMA

# margin engine per tile within each 16-tile chunk ("D" DVE stt /
# "A" scalar engine + tensor engine); "A" count per chunk must be even
# (pairs share one bf16 relu tile for the ones-matmul)
PATTERN = list(("A" * 12 + "D" * 4) * 6 + "A" * 8 + "D" * 8 + "D" * 16)

_NC_CACHE = {}
LAST_RESULTS = None
SPLIT_WAITS = True   # off for CoreSim (its event loop rejects bare Drains)
TRACE = False
TRACE_KW = {}
DEBUG_G = False      # add a gout output carrying the gathered G tile


def _split_multi_waits(nc):
    for f in nc.m.functions:
        for blk in f.blocks:
            insts = list(blk.instructions)
            out = []
            changed = False
            for inst in insts:
                si = inst.sync_info
                if si is not None and si.on_wait is not None and len(si.on_wait) > 1:
                    waits = list(si.on_wait)
                    for w in waits[:-1]:
                        d = mybir.InstDrain(
                            name=nc.get_next_instruction_name(),
                            ins=[], outs=[], bass_is_fusable=False)
                        d.engine = inst.engine
                        d.sync_info = mybir.SyncInfo(on_wait=[w], on_update=[])
                        out.append(d)
                    inst.sync_info = mybir.SyncInfo(
                        on_wait=[waits[-1]], on_update=list(si.on_update or []))
                    changed = True
                out.append(inst)
            if changed:
                blk.instructions = out


def _assignment():
    assert N_TILES % len(PATTERN) == 0
    return [PATTERN[t % len(PATTERN)] for t in range(N_TILES)]  # full or cyclic


def build_nc():
    key = (tuple(_assignment()), SPLIT_WAITS, DEBUG_G)
    if key in _NC_CACHE:
        return _NC_CACHE[key]

    assign = _assignment()
    a_tiles = [t for t, c in enumerate(assign) if c == "A"]
    n_a = len(a_tiles)
    for c in range(N_CHUNKS):
        n_ac = sum(1 for t in range(c * TILES_PER_DMA, (c + 1) * TILES_PER_DMA)
                   if assign[t] == "A")
        assert n_ac % 2 == 0, "A count per chunk must be even"

    nc = bass.Bass()
    x = nc.declare_dram_parameter("x", [B_LOCAL, C], F32, isOutput=False)
    lbl = nc.declare_dram_parameter("lbl", [P, N_TILES], F32, isOutput=False)
    out = nc.declare_dram_parameter("out", [1, 1], F32, isOutput=True)
    gout = (nc.declare_dram_parameter("gout", [P, N_TILES], F32, isOutput=True)
            if DEBUG_G else None)

    # row r = p*128 + t  ->  partition p, tile t
    xv = x.rearrange("(p t) c -> p (t c)", p=P, t=N_TILES)

    with tile.TileContext(nc) as tc, ExitStack() as ctx:
        singles = ctx.enter_context(tc.tile_pool(name="singles", bufs=1))
        xpool = ctx.enter_context(tc.tile_pool(name="x", bufs=4))
        scr = ctx.enter_context(tc.tile_pool(name="scr", bufs=4))
        prs = ctx.enter_context(tc.tile_pool(name="prs", bufs=4))
        sgp = ctx.enter_context(tc.tile_pool(name="sg", bufs=6))
        psum = ctx.enter_context(tc.tile_pool(name="psum", bufs=1, space="PSUM"))

        lbl_sb = singles.tile([P, N_TILES], F32)
        nc.sync.dma_start(lbl_sb[:], lbl[:])

        HMB = TILES_PER_DMA // 2 * C   # half-chunk elements per partition

        def issue_xw(chunk, splits=2):
            xw = xpool.tile([P, TILES_PER_DMA * C], F32, name="xw")
            base = chunk * TILES_PER_DMA * C
            step = TILES_PER_DMA * C // splits
            for h2 in range(splits):
                nc.sync.dma_start(
                    xw[:, h2 * step:(h2 + 1) * step],
                    xv[:, base + h2 * step:base + (h2 + 1) * step])
            return xw

        xw0 = issue_xw(0, splits=4)

        iota_i = singles.tile([P, C], mybir.dt.int32)
        nc.gpsimd.iota(iota_i[:], pattern=[[1, C]], base=0, channel_multiplier=0)
        iota_f = singles.tile([P, C], F32)
        nc.vector.tensor_copy(iota_f[:], iota_i[:])

        ones = singles.tile([P, 1], F32)
        nc.vector.memset(ones[:], 1.0)
        warm = singles.tile([P, 1], F32)
        nc.scalar.activation(warm[:], ones[:],
                             mybir.ActivationFunctionType.Sign,
                             bias=0.0, scale=1.0)

        G = singles.tile([P, N_TILES], F32)
        M = singles.tile([P, N_TILES], F32)      # D-tile margins; A cols = 0
        if n_a:
            ones_bf = singles.tile([P, 1], BF16)
            nc.vector.memset(ones_bf[:], 1.0)
            nc.vector.memset(M[:], 0.0)
            NGB = singles.tile([P, N_TILES], BF16)  # -g: ACT bias + weights
            neg255 = singles.tile([P, 1], BF16)
            nc.vector.memset(neg255[:], -255.0)
            ps_r = psum.tile([P, 512], F32, tag="ps_r")
            ps_s = [psum.tile([P, 512], F32, tag=f"ps_s{i}", name=f"ps_s{i}")
                    for i in range(2)]

        mm_r = 0
        mm_s = [0, 0]
        a_seen = 0
        rpair = None
        for chunk in range(N_CHUNKS):
            xw = xw0 if chunk == 0 else issue_xw(chunk)
            HALF = TILES_PER_DMA // 2
            d_tiles = []
            for h in range(2):
                ks = range(h * HALF, (h + 1) * HALF)
                for kk in ks:
                    t = chunk * TILES_PER_DMA + kk
                    xb = xw[:, kk * C:(kk + 1) * C]
                    sel = scr.tile([P, C], F32, tag="sel", name="sel")
                    nc.vector.scalar_tensor_tensor(
                        out=sel[:], in0=iota_f[:], scalar=lbl_sb[:, t:t + 1],
                        in1=xb,
                        op0=mybir.AluOpType.is_equal, op1=mybir.AluOpType.mult,
                        accum_out=G[:, t:t + 1])
                hcols = slice(chunk * TILES_PER_DMA + h * HALF,
                              chunk * TILES_PER_DMA + (h + 1) * HALF)
                if n_a:
                    nc.vector.tensor_scalar_mul(NGB[:, hcols], G[:, hcols], -1.0)
                for kk in ks:
                    t = chunk * TILES_PER_DMA + kk
                    if assign[t] == "D":
                        d_tiles.append(kk)
                        continue
                    xb = xw[:, kk * C:(kk + 1) * C]
                    gc = G[:, t:t + 1]
                    u = a_seen % 2
                    if u == 0:
                        rpair = prs.tile([P, 2 * C], BF16, tag="rpair",
                                         name="rpair")
                    rb = rpair[:, u * C:(u + 1) * C]
                    # sign(g - x) = -sign(x - g): bias is the raw G column;
                    # (-g)^T @ sign(g-x) = g^T @ sign(x-g), +0.5 in tail.
                    sg = sgp.tile([P, C], BF16, tag="sg", name="sg")
                    nc.scalar.activation(
                        sg[:], xb, mybir.ActivationFunctionType.Sign,
                        bias=gc, scale=-1.0)
                    nc.scalar.activation(
                        rb, xb, mybir.ActivationFunctionType.Relu,
                        bias=NGB[:, t:t + 1], scale=1.0)
                    nc.tensor.matmul(ps_s[u][:1, :C], NGB[:, t:t + 1], sg[:],
                                     start=(mm_s[u] == 0),
                                     stop=(mm_s[u] == n_a // 2 - 1))
                    # 255*sum(g) into col C of the same bank (x0.5 in tail
                    # gives the +127.5*sum_A g count correction)
                    nc.tensor.matmul(ps_s[u][:1, C:C + 1], NGB[:, t:t + 1],
                                     neg255[:],
                                     start=(mm_s[u] == 0),
                                     stop=(mm_s[u] == n_a // 2 - 1))
                    mm_s[u] += 1
                    if u == 1:
                        nc.tensor.matmul(ps_r[:1, :], ones_bf[:], rpair[:],
                                         start=(mm_r == 0),
                                         stop=(mm_r == n_a // 2 - 1))
                        mm_r += 1
                    a_seen += 1
            for kk in d_tiles:
                t = chunk * TILES_PER_DMA + kk
                xb = xw[:, kk * C:(kk + 1) * C]
                gc = G[:, t:t + 1]
                mp = scr.tile([P, C], F32, tag="mp", name="mp")
                nc.vector.scalar_tensor_tensor(
                    out=mp[:], in0=xb, scalar=gc, in1=xb,
                    op0=mybir.AluOpType.is_gt, op1=mybir.AluOpType.mult,
                    accum_out=M[:, t:t + 1])

        # ---- tail ------------------------------------------------------
        tmp = scr.tile([P, N_TILES], F32, tag="tail", name="tmp")
        nc.vector.tensor_scalar_add(tmp[:], G[:], 0.1)
        inv = scr.tile([P, N_TILES], F32, tag="tail2", name="inv")
        nc.vector.reciprocal(inv[:], tmp[:])
        tot = scr.tile([P, N_TILES], F32, tag="tail3", name="tot")
        nc.vector.tensor_tensor(out=tot[:], in0=inv[:], in1=M[:],
                                op=mybir.AluOpType.add)
        rows = singles.tile([P, 1], F32)
        nc.vector.tensor_reduce(rows[:], tot[:], axis=mybir.AxisListType.X,
                                op=mybir.AluOpType.add)
        ps_fin = psum.tile([P, 8], F32, tag="fin")
        nc.tensor.matmul(ps_fin[:1, :1], ones[:], rows[:])

        fin = singles.tile([1, 1], F32)
        nc.vector.tensor_copy(fin[:], ps_fin[:1, :1])
        acc_terms = [fin]
        if n_a:
            # + sum(ps_r) - 0.5*sum(ps_s0 + ps_s1)
            cb = singles.tile([1, 1026], F32)
            nc.vector.tensor_copy(cb[:, 0:512], ps_r[:1, :])
            nc.vector.tensor_copy(cb[:, 512:769], ps_s[0][:1, :C + 1])
            nc.vector.tensor_copy(cb[:, 769:1026], ps_s[1][:1, :C + 1])
            tot1 = singles.tile([1, 1], F32)
            nc.vector.tensor_reduce(tot1[:], cb[:, 0:512],
                                    axis=mybir.AxisListType.X,
                                    op=mybir.AluOpType.add)
            # ps_s cols 0..C-1 carry (-g)*sign(g-x) = g*sign(x-g) sums and
            # col C carries 255*sum_A g; margin needs +(both)/2
            sc = singles.tile([1, 514], F32)
            nc.vector.tensor_scalar(out=sc[:], in0=cb[:, 512:1026],
                                    scalar1=0.5, scalar2=None,
                                    op0=mybir.AluOpType.mult)
            tot2 = singles.tile([1, 1], F32)
            nc.vector.tensor_reduce(tot2[:], sc[:],
                                    axis=mybir.AxisListType.X,
                                    op=mybir.AluOpType.add)
            acc_terms += [tot1, tot2]
        res = acc_terms[0]
        for ti, term in enumerate(acc_terms[1:]):
            nxt = singles.tile([1, 1], F32, name=f"sumchain{ti}")
            nc.vector.tensor_tensor(out=nxt[:], in0=res[:], in1=term[:],
                                    op=mybir.AluOpType.add)
            res = nxt
        nc.sync.dma_start(out[:], res[:])
        if gout is not None:
            gcp = singles.tile([P, N_TILES], F32, name="gcp")
            nc.vector.tensor_copy(gcp[:], G[:])
            nc.sync.dma_start(gout[:], gcp[:])

    if SPLIT_WAITS:
        _split_multi_waits(nc)
    _NC_CACHE[key] = nc
    return nc


def _prep_inputs(rna_cell_out, rna_cell_label):
    x = np.ascontiguousarray(np.asarray(rna_cell_out, dtype=np.float32))
    l = np.asarray(rna_cell_label).astype(np.int64)
    assert x.shape == (B, C) and l.shape == (B,)
    in_maps = []
    for i in range(N_CORES):
        xs = x[i * B_LOCAL:(i + 1) * B_LOCAL]
        ls = l[i * B_LOCAL:(i + 1) * B_LOCAL]
        lbl = ls.reshape(P, N_TILES).astype(np.float32)
        in_maps.append({"x": xs, "lbl": np.ascontiguousarray(lbl)})
    return in_maps


def kernel(rna_cell_out, rna_cell_label):
    global LAST_RESULTS
    nc = build_nc()
    in_maps = _prep_inputs(rna_cell_out, rna_cell_label)
    res = run_bass_kernel_spmd(nc, in_maps, list(range(N_CORES)),
                               trace=TRACE, **TRACE_KW)
    LAST_RESULTS = res
    parts = [float(res.results[i]["out"][0, 0]) for i in range(N_CORES)]
    loss = np.float32(np.sum(np.array(parts, dtype=np.float64)) / B)
    return np.array([loss], dtype=np.float32)


# revision 17
# speedup vs baseline: 1.0593x; 1.0009x over previous
"""Trainium2 Bass kernel for the CellLoss problem.

loss = mean_i [ 1/(x[i, l_i] + 0.1) + sum_j x[i,j] * (x[i,j] > x[i, l_i]) ]
with x: [131072, 256] f32, l: [131072] int labels in [0, 256).

Pure data parallel across 8 NeuronCores (16384 rows each). Per core,
partition p owns rows [p*128, (p+1)*128); tile t is the [128, 256]
block of row p*128+t per partition.

Structure (per 16-tile chunk, streamed in two 1 MiB DMA halves):
  gather (DVE): per tile one fused stt (iota==l)*x with a per-row sum
      accumulator into G. Emitted in 8-tile half-batches so the scalar
      engine can start each chunk's work after half the gathers.
  margin, two engine variants cycled by PATTERN per 16-tile chunk:
   "D": DVE stt, (x is_gt g) mult x with row-sum accumulator into M.
   "A": scalar-engine Relu(x-g) and Sign(g-x) passes writing bf16 tiles;
      the tensor engine accumulates GLOBAL sums in PSUM: ones^T @
      relu-pairs, and gb^T @ sign(g-x) per tile (gb = bf16(g), so the
      matmuls run at bf16 rate, 4x the f32 rate). Using
      sum_i g_i*cnt_i = (-sum g*sign(g-x) + 255*sum g)/2 (sign(0)=0 at
      the label), the margin needs only these global sums.
Tail: inv = 1/(G+0.1); per-row totals + the A-tile 127.5*g correction;
partition sum via ones-matmul; one f32 partial per core; the host sums
the 8 partials and divides by B.

bf16 is used ONLY for relu magnitudes (unbiased rounding, ~1e-6 effect),
the sign values (exact), and the g matmul weights for the count term
(~1e-5 effect); G itself stays exact f32 for the inv term and compares.
A-tiles lead each chunk (pattern head) and NG/GB bias/weight prep is
batched per half-chunk, keeping the scalar engine fed while DVE runs
the gathers; a warm-up Sign activation pulls the 1.3 us activation
table load into the startup DMA window.

This walrus accepts one sync wait per instruction; Tile can emit
several. _split_multi_waits() hoists extras onto Drain carriers.
"""

import numpy as np
from contextlib import ExitStack

import concourse.bass as bass
import concourse.mybir as mybir
import concourse.tile as tile
from concourse.bass_utils import run_bass_kernel_spmd

F32 = mybir.dt.float32
BF16 = mybir.dt.bfloat16

B, C = 131072, 256
N_CORES = 8
B_LOCAL = B // N_CORES          # 16384
P = 128
N_TILES = B_LOCAL // P          # 128
TILES_PER_DMA = 16              # [128, 4096] f32 = 2 MiB per DMA
N_CHUNKS = N_TILES // TILES_PER_DMA

# margin engine per tile within each 16-tile chunk ("D" DVE stt /
# "A" scalar engine + tensor engine); "A" count per chunk must be even
# (pairs share one bf16 relu tile for the ones-matmul)
PATTERN = list(("A" * 12 + "D" * 4) * 6 + "A" * 8 + "D" * 8 + "D" * 16)

_NC_CACHE = {}
LAST_RESULTS = None
SPLIT_WAITS = True   # off for CoreSim (its event loop rejects bare Drains)
TRACE = False
TRACE_KW = {}
DEBUG_G = False      # add a gout output carrying the gathered G tile


def _split_multi_waits(nc):
    for f in nc.m.functions:
        for blk in f.blocks:
            insts = list(blk.instructions)
            out = []
            changed = False
            for inst in insts:
                si = inst.sync_info
                if si is not None and si.on_wait is not None and len(si.on_wait) > 1:
                    waits = list(si.on_wait)
                    for w in waits[:-1]:
                        d = mybir.InstDrain(
                            name=nc.get_next_instruction_name(),
                            ins=[], outs=[], bass_is_fusable=False)
                        d.engine = inst.engine
                        d.sync_info = mybir.SyncInfo(on_wait=[w], on_update=[])
                        out.append(d)
                    inst.sync_info = mybir.SyncInfo(
                        on_wait=[waits[-1]], on_update=list(si.on_update or []))
                    changed = True
                out.append(inst)
            if changed:
                blk.instructions = out


def _assignment():
    assert N_TILES % len(PATTERN) == 0
    return [PATTERN[t % len(PATTERN)] for t in range(N_TILES)]  # full or cyclic


def build_nc():
    key = (tuple(_assignment()), SPLIT_WAITS, DEBUG_G)
    if key in _NC_CACHE:
        return _NC_CACHE[key]

    assign = _assignment()
    a_tiles = [t for t, c in enumerate(assign) if c == "A"]
    n_a = len(a_tiles)
    for c in range(N_CHUNKS):
        n_ac = sum(1 for t in range(c * TILES_PER_DMA, (c + 1) * TILES_PER_DMA)
                   if assign[t] == "A")
        assert n_ac % 2 == 0, "A count per chunk must be even"

    nc = bass.Bass()
    x = nc.declare_dram_parameter("x", [B_LOCAL, C], F32, isOutput=False)
    lbl = nc.declare_dram_parameter("lbl", [P, N_TILES], F32, isOutput=False)
    out = nc.declare_dram_parameter("out", [1, 1], F32, isOutput=True)
    gout = (nc.declare_dram_parameter("gout", [P, N_TILES], F32, isOutput=True)
            if DEBUG_G else None)

    # row r = p*128 + t  ->  partition p, tile t
    xv = x.rearrange("(p t) c -> p (t c)", p=P, t=N_TILES)

    with tile.TileContext(nc) as tc, ExitStack() as ctx:
        singles = ctx.enter_context(tc.tile_pool(name="singles", bufs=1))
        xpool = ctx.enter_context(tc.tile_pool(name="x", bufs=4))
        scr = ctx.enter_context(tc.tile_pool(name="scr", bufs=4))
        prs = ctx.enter_context(tc.tile_pool(name="prs", bufs=4))
        sgp = ctx.enter_context(tc.tile_pool(name="sg", bufs=6))
        psum = ctx.enter_context(tc.tile_pool(name="psum", bufs=1, space="PSUM"))

        lbl_sb = singles.tile([P, N_TILES], F32)
        nc.sync.dma_start(lbl_sb[:], lbl[:])

        HMB = TILES_PER_DMA // 2 * C   # half-chunk elements per partition

        def issue_xw(chunk, splits=2):
            xw = xpool.tile([P, TILES_PER_DMA * C], F32, name="xw")
            base = chunk * TILES_PER_DMA * C
            step = TILES_PER_DMA * C // splits
            for h2 in range(splits):
                nc.sync.dma_start(
                    xw[:, h2 * step:(h2 + 1) * step],
                    xv[:, base + h2 * step:base + (h2 + 1) * step])
            return xw

        xw0 = issue_xw(0, splits=4)

        iota_i = singles.tile([P, C], mybir.dt.int32)
        nc.gpsimd.iota(iota_i[:], pattern=[[1, C]], base=0, channel_multiplier=0)
        iota_f = singles.tile([P, C], F32)
        nc.vector.tensor_copy(iota_f[:], iota_i[:])

        ones = singles.tile([P, 1], F32)
        nc.vector.memset(ones[:], 1.0)
        warm = singles.tile([P, 1], F32)
        nc.scalar.activation(warm[:], ones[:],
                             mybir.ActivationFunctionType.Sign,
                             bias=0.0, scale=1.0)

        G = singles.tile([P, N_TILES], F32)
        M = singles.tile([P, N_TILES], F32)      # D-tile margins; A cols = 0
        if n_a:
            ones_bf = singles.tile([P, 1], BF16)
            nc.vector.memset(ones_bf[:], 1.0)
            nc.vector.memset(M[:], 0.0)
            NGB = singles.tile([P, N_TILES], BF16)  # -g: ACT bias + weights
            neg255 = singles.tile([P, 1], BF16)
            nc.vector.memset(neg255[:], -255.0)
            ps_r = psum.tile([P, 512], F32, tag="ps_r")
            ps_s = [psum.tile([P, 512], F32, tag=f"ps_s{i}", name=f"ps_s{i}")
                    for i in range(2)]
            ps_c = [psum.tile([P, 8], F32, tag=f"ps_c{i}", name=f"ps_c{i}")
                    for i in range(2)]

        mm_r = 0
        mm_s = [0, 0]
        a_seen = 0
        rpair = None
        for chunk in range(N_CHUNKS):
            xw = xw0 if chunk == 0 else issue_xw(chunk)
            HALF = TILES_PER_DMA // 2
            d_tiles = []
            for h in range(2):
                ks = range(h * HALF, (h + 1) * HALF)
                for kk in ks:
                    t = chunk * TILES_PER_DMA + kk
                    xb = xw[:, kk * C:(kk + 1) * C]
                    sel = scr.tile([P, C], F32, tag="sel", name="sel")
                    nc.vector.scalar_tensor_tensor(
                        out=sel[:], in0=iota_f[:], scalar=lbl_sb[:, t:t + 1],
                        in1=xb,
                        op0=mybir.AluOpType.is_equal, op1=mybir.AluOpType.mult,
                        accum_out=G[:, t:t + 1])
                hcols = slice(chunk * TILES_PER_DMA + h * HALF,
                              chunk * TILES_PER_DMA + (h + 1) * HALF)
                if n_a:
                    nc.vector.tensor_scalar_mul(NGB[:, hcols], G[:, hcols], -1.0)
                for kk in ks:
                    t = chunk * TILES_PER_DMA + kk
                    if assign[t] == "D":
                        d_tiles.append(kk)
                        continue
                    xb = xw[:, kk * C:(kk + 1) * C]
                    gc = G[:, t:t + 1]
                    u = a_seen % 2
                    if u == 0:
                        rpair = prs.tile([P, 2 * C], BF16, tag="rpair",
                                         name="rpair")
                    rb = rpair[:, u * C:(u + 1) * C]
                    # sign(g - x) = -sign(x - g): bias is the raw G column;
                    # (-g)^T @ sign(g-x) = g^T @ sign(x-g), +0.5 in tail.
                    sg = sgp.tile([P, C], BF16, tag="sg", name="sg")
                    nc.scalar.activation(
                        sg[:], xb, mybir.ActivationFunctionType.Sign,
                        bias=gc, scale=-1.0)
                    nc.scalar.activation(
                        rb, xb, mybir.ActivationFunctionType.Relu,
                        bias=NGB[:, t:t + 1], scale=1.0)
                    nc.tensor.matmul(ps_s[u][:1, :C], NGB[:, t:t + 1], sg[:],
                                     start=(mm_s[u] == 0),
                                     stop=(mm_s[u] == n_a // 2 - 1))
                    # 255*sum(g) into its own bank (x0.5 in tail gives
                    # the +127.5*sum_A g count correction)
                    nc.tensor.matmul(ps_c[u][:1, :1], NGB[:, t:t + 1],
                                     neg255[:],
                                     start=(mm_s[u] == 0),
                                     stop=(mm_s[u] == n_a // 2 - 1))
                    mm_s[u] += 1
                    if u == 1:
                        nc.tensor.matmul(ps_r[:1, :], ones_bf[:], rpair[:],
                                         start=(mm_r == 0),
                                         stop=(mm_r == n_a // 2 - 1))
                        mm_r += 1
                    a_seen += 1
            for kk in d_tiles:
                t = chunk * TILES_PER_DMA + kk
                xb = xw[:, kk * C:(kk + 1) * C]
                gc = G[:, t:t + 1]
                mp = scr.tile([P, C], F32, tag="mp", name="mp")
                nc.vector.scalar_tensor_tensor(
                    out=mp[:], in0=xb, scalar=gc, in1=xb,
                    op0=mybir.AluOpType.is_gt, op1=mybir.AluOpType.mult,
                    accum_out=M[:, t:t + 1])

        # ---- tail ------------------------------------------------------
        tmp = scr.tile([P, N_TILES], F32, tag="tail", name="tmp")
        nc.vector.tensor_scalar_add(tmp[:], G[:], 0.1)
        inv = scr.tile([P, N_TILES], F32, tag="tail2", name="inv")
        nc.vector.reciprocal(inv[:], tmp[:])
        tot = scr.tile([P, N_TILES], F32, tag="tail3", name="tot")
        nc.vector.tensor_tensor(out=tot[:], in0=inv[:], in1=M[:],
                                op=mybir.AluOpType.add)
        rows = singles.tile([P, 1], F32)
        nc.vector.tensor_reduce(rows[:], tot[:], axis=mybir.AxisListType.X,
                                op=mybir.AluOpType.add)
        ps_fin = psum.tile([P, 8], F32, tag="fin")
        nc.tensor.matmul(ps_fin[:1, :1], ones[:], rows[:])

        fin = singles.tile([1, 1], F32)
        nc.vector.tensor_copy(fin[:], ps_fin[:1, :1])
        acc_terms = [fin]
        if n_a:
            # + sum(ps_r) - 0.5*sum(ps_s0 + ps_s1)
            cb = singles.tile([1, 1026], F32)
            nc.vector.tensor_copy(cb[:, 0:512], ps_r[:1, :])
            nc.vector.tensor_copy(cb[:, 512:768], ps_s[0][:1, :C])
            nc.vector.tensor_copy(cb[:, 768:769], ps_c[0][:1, :1])
            nc.vector.tensor_copy(cb[:, 769:1025], ps_s[1][:1, :C])
            nc.vector.tensor_copy(cb[:, 1025:1026], ps_c[1][:1, :1])
            tot1 = singles.tile([1, 1], F32)
            nc.vector.tensor_reduce(tot1[:], cb[:, 0:512],
                                    axis=mybir.AxisListType.X,
                                    op=mybir.AluOpType.add)
            # ps_s cols 0..C-1 carry (-g)*sign(g-x) = g*sign(x-g) sums and
            # col C carries 255*sum_A g; margin needs +(both)/2
            sc = singles.tile([1, 514], F32)
            nc.vector.tensor_scalar(out=sc[:], in0=cb[:, 512:1026],
                                    scalar1=0.5, scalar2=None,
                                    op0=mybir.AluOpType.mult)
            tot2 = singles.tile([1, 1], F32)
            nc.vector.tensor_reduce(tot2[:], sc[:],
                                    axis=mybir.AxisListType.X,
                                    op=mybir.AluOpType.add)
            acc_terms += [tot1, tot2]
        res = acc_terms[0]
        for ti, term in enumerate(acc_terms[1:]):
            nxt = singles.tile([1, 1], F32, name=f"sumchain{ti}")
            nc.vector.tensor_tensor(out=nxt[:], in0=res[:], in1=term[:],
                                    op=mybir.AluOpType.add)
            res = nxt
        nc.sync.dma_start(out[:], res[:])
        if gout is not None:
            gcp = singles.tile([P, N_TILES], F32, name="gcp")
            nc.vector.tensor_copy(gcp[:], G[:])
            nc.sync.dma_start(gout[:], gcp[:])

    if SPLIT_WAITS:
        _split_multi_waits(nc)
    _NC_CACHE[key] = nc
    return nc


def _prep_inputs(rna_cell_out, rna_cell_label):
    x = np.ascontiguousarray(np.asarray(rna_cell_out, dtype=np.float32))
    l = np.asarray(rna_cell_label).astype(np.int64)
    assert x.shape == (B, C) and l.shape == (B,)
    in_maps = []
    for i in range(N_CORES):
        xs = x[i * B_LOCAL:(i + 1) * B_LOCAL]
        ls = l[i * B_LOCAL:(i + 1) * B_LOCAL]
        lbl = ls.reshape(P, N_TILES).astype(np.float32)
        in_maps.append({"x": xs, "lbl": np.ascontiguousarray(lbl)})
    return in_maps


def kernel(rna_cell_out, rna_cell_label):
    global LAST_RESULTS
    nc = build_nc()
    in_maps = _prep_inputs(rna_cell_out, rna_cell_label)
    res = run_bass_kernel_spmd(nc, in_maps, list(range(N_CORES)),
                               trace=TRACE, **TRACE_KW)
    LAST_RESULTS = res
    parts = [float(res.results[i]["out"][0, 0]) for i in range(N_CORES)]
    loss = np.float32(np.sum(np.array(parts, dtype=np.float64)) / B)
    return np.array([loss], dtype=np.float32)
